# revision 5
# baseline (speedup 1.0000x reference)
"""Trainium2 Bass kernel for GeneRegulatoryNetwork pairwise regulatory matrix.

reg[i,j] = sign(argmax(MLP(cat[x_i,x_j]))) * (x_i^T Wb x_j + bb), zero diag.

Decomposition (verified vs reference):
  Ai = X @ W1[:, :h].T            (per-gene i contribution)
  Bj = X @ W1[:, h:].T + b1       (per-gene j contribution, b1 folded)
  hid(i,j) = relu(Ai[i] + Bj[j])               [h]
  p = hid . u ; q = hid . v                    (u = W2[0]-W2[1], v = W2[0]-W2[2])
  Sign closed form (matches first-max argmax semantics exactly), with
  P = p+pb, Q = q+qb (pb = b2[0]-b2[1], qb = b2[0]-b2[2]):
      m2  = min(P, 0) ; hp1 = 1[P >= 0] + 1
      r   = Q - m2                   (r >= 0  <=>  NOT class2)
      g2  = 1[r >= 0] * hp1          (in {0, 1, 2})
      reg = (g2 - 1) * (aff + bb)
  aff[j,i] = xt[:,jblk].T @ y2  with y2 = Wb0.T @ Xm.T (host-precomputed)

v3 design (cost-model driven):
  - HOST precomputes bjT (fp16 + fp32, b1 folded), aiT, y2: no device
    preamble matmuls/drains.  Device work = 96 hid ops + tiny PE matmuls
    + postprocess.
  - hid row split D/A/G ~ 60/15/21 balances engine busy (260/825/640 ns
    per row).  DVE + Pool rows read fp16 bjT (DVE 4x mode); ACT rows read
    fp32 bjT (dtype-neutral cost) to keep the rel-err margin.
  - Non-last chunks: ONE unbiased ACT deint of the interleaved P/Q PSUM
    block (biases folded into the Pool chain's scalar slots) + ACT affs
    drain + 7-op Pool chain.  DVE does nothing here.
  - Last chunk: all-D rows + all-DVE direct-PSUM chain (shortest tail).
  - Postprocess + output DMAs emitted under tc.high_priority() so ready
    chain ops preempt queued hid ops (kills the chunk-post latency drift).
  - Inputs staged over 3 parallel engine DMA queues (SP / ACT / Pool).
"""

import os as _os
import sys

if "/opt/trn_rl_repo" not in sys.path:
    sys.path.insert(0, "/opt/trn_rl_repo")

import numpy as np

N = 768
H = 128
NCORES = 8
R = N // NCORES  # 96 rows per core
JB = N // H      # 6 j-blocks of 128
S = JB * R       # 576 (b, i) slots

# i-chunk sizes for postprocess (last chunk small + all-DVE -> short tail)
CHUNKS = [int(x) for x in _os.environ.get("BASS_CHUNKS", "22,22,22,22,8").split(",")]
assert sum(CHUNKS) == R
CH_OFF = [sum(CHUNKS[:k]) for k in range(len(CHUNKS))]
# engine split for the hid ops (DVE / ACT / GPSIMD; DVE gets the rest)
ND_A = int(_os.environ.get("BASS_NA", "15"))
ND_G = int(_os.environ.get("BASS_NG", "21"))

_NC_CACHE = {}


def _engine_pattern():
    """Static i -> engine map from {"D", "A", "G"}.

    Chunks 0..n-2 get a weighted interleave of D/A/G, reordered within each
    chunk as [A..., G..., D...] so ACT rows start early and the chunk's last
    row is a fast DVE op (the chunk post can start right after).  The last
    chunk is all-D: its rows + the tail chain run back-to-back on DVE while
    ACT/Pool finish the previous chunk's postprocess.
    """
    lci = CHUNKS[-1]
    body = R - lci
    nd0 = body - ND_A - ND_G
    assert nd0 >= 0
    counts = {"A": ND_A, "G": ND_G, "D": nd0}
    acc = {"A": 0.0, "G": 0.0, "D": 0.0}
    pat = []
    for _ in range(body):
        for e in counts:
            acc[e] += counts[e] / body
        e = max(acc, key=lambda k: acc[k])
        acc[e] -= 1.0
        pat.append(e)
    out = []
    for c in range(len(CHUNKS) - 1):
        seg = pat[CH_OFF[c] : CH_OFF[c] + CHUNKS[c]]
        out += ["A"] * seg.count("A") + ["G"] * seg.count("G") + ["D"] * seg.count("D")
    out += ["D"] * lci
    return out


def build_nc():
    key = (ND_A, ND_G, tuple(CHUNKS))
    if key in _NC_CACHE:
        return _NC_CACHE[key]
    from contextlib import ExitStack

    import concourse.bass as bass
    import concourse.tile as tile
    from concourse import bacc, mybir

    f32 = mybir.dt.float32
    fp16 = mybir.dt.float16
    Alu = mybir.AluOpType
    Relu = mybir.ActivationFunctionType.Relu
    Ident = mybir.ActivationFunctionType.Identity

    nc = bacc.Bacc("TRN2", target_bir_lowering=False, debug=False)

    # bj16: [bjT fp16 (768) | uv fp16 (2)]
    d_bj = nc.dram_tensor("bj16", [H, N + 2], fp16, kind="ExternalInput").ap()
    # fa: [aiT f32 (96) | pbc | npbc | qbc | nqbc | bbc | uv32 (2)]
    FA_W = R + 5 + 2
    d_fa = nc.dram_tensor("fa", [H, FA_W], f32, kind="ExternalInput").ap()
    d_bj32 = nc.dram_tensor("bj32", [H, N], f32, kind="ExternalInput").ap()
    d_xt = nc.dram_tensor("xt", [H, N], f32, kind="ExternalInput").ap()
    d_y2 = nc.dram_tensor("y2", [H, R], f32, kind="ExternalInput").ap()
    outT = nc.dram_tensor("outT", [N, R], f32, kind="ExternalOutput").ap()

    pat = _engine_pattern()

    with tile.TileContext(nc) as tc, ExitStack() as ctx:
        const = ctx.enter_context(tc.tile_pool(name="const", bufs=1))
        work = ctx.enter_context(tc.tile_pool(name="work", bufs=1))
        hidp = ctx.enter_context(
            tc.tile_pool(name="hid", bufs=int(_os.environ.get("BASS_HBUF", "20"))))
        pspq = ctx.enter_context(tc.tile_pool(name="pspq", bufs=1, space="PSUM"))
        psaf = ctx.enter_context(tc.tile_pool(name="psaf", bufs=1, space="PSUM"))

        pq_ps = pspq.tile([H, 2 * S], f32, tag="pq")       # [j, (c, b, i, 2)]
        aff_ps = psaf.tile([H, S], f32, tag="aff")          # [j, (c, b, i)]

        bj_sb = const.tile([H, N + 2], fp16, tag="bj")
        fa_sb = const.tile([H, FA_W], f32, tag="fa")
        bj32_sb = const.tile([H, N], f32, tag="bj32")
        xt_sb = const.tile([H, N], f32, tag="xt")
        y2_sb = const.tile([H, R], f32, tag="y2")

        # ---- input DMAs on 3 parallel engine queues (SP / ACT / Pool) ----
        nc.sync.dma_start(bj_sb[:], d_bj[:])                      # arr ~2.31us
        nc.scalar.dma_start(fa_sb[:], d_fa[:])                    # arr ~2.22us
        nc.scalar.dma_start(bj32_sb[:, 0:384], d_bj32[:, 0:384])  # arr ~2.81us
        nc.gpsimd.dma_start(bj32_sb[:, 384:N], d_bj32[:, 384:N])  # arr ~2.48us
        nc.sync.dma_start(xt_sb[:], d_xt[:])                      # arr ~3.50us
        nc.gpsimd.dma_start(y2_sb[:], d_y2[:])                    # arr ~2.98us

        # ---- t=0: trigger the ACT table load during the DMA wait ----
        tw = const.tile([H, 1], f32, tag="tw")
        nc.vector.memset(tw[:], 0.25)
        tact = const.tile([H, 1], f32, tag="tact")
        nc.scalar.activation(tact[:], tw[:], Relu, bias=0.0)

        bjT16 = bj_sb[:, 0:N]
        uv16 = bj_sb[:, N : N + 2]
        aiT = fa_sb[:, 0:R]
        pb_sb = fa_sb[:, R : R + 1]
        npb_sb = fa_sb[:, R + 1 : R + 2]
        qb_sb = fa_sb[:, R + 2 : R + 3]
        nqb_sb = fa_sb[:, R + 3 : R + 4]
        bb_sb = fa_sb[:, R + 4 : R + 5]
        uv32 = fa_sb[:, R + 5 : R + 7]

        # ---- main loop ----
        affs_last = None
        c = 0
        for i in range(R):
            while i >= CH_OFF[c] + CHUNKS[c]:
                c += 1
            il = i - CH_OFF[c]
            ci = CHUNKS[c]
            e = pat[i]
            if e == "A":
                hid = hidp.tile([H, N], f32, tag="hid")
                nc.scalar.activation(hid[:], bj32_sb[:], Relu,
                                     bias=aiT[:, i : i + 1])
                uv_mm = uv32
            else:
                hid = hidp.tile([H, N], fp16, tag="hid")
                if e == "D":
                    nc.vector.tensor_scalar(hid[:], bjT16, aiT[:, i : i + 1],
                                            0.0, Alu.add, Alu.max)
                else:
                    nc.gpsimd.tensor_scalar(hid[:], bjT16, aiT[:, i : i + 1],
                                            0.0, Alu.add, Alu.max)
                uv_mm = uv16
            for b in range(JB):
                o = 2 * (JB * CH_OFF[c] + b * ci + il)
                nc.tensor.matmul(pq_ps[:, o : o + 2], hid[:, b * H : (b + 1) * H],
                                 uv_mm, start=True, stop=True)

            if i == CH_OFF[2]:
                # last chunk's affinity (+bb) computed mid-loop: PE and ACT
                # both have slack here, so the final chunk's reg op only
                # needs cheap SBUF stt's in the tail
                lc = len(CHUNKS) - 1
                lci = CHUNKS[lc]
                lcoff = JB * CH_OFF[lc]
                for b in range(JB):
                    ao = lcoff + b * lci
                    nc.tensor.matmul(aff_ps[:, ao : ao + lci],
                                     xt_sb[:, b * H : (b + 1) * H],
                                     y2_sb[:, CH_OFF[lc] : CH_OFF[lc] + lci],
                                     start=True, stop=True)
                affs_last = work.tile([H, JB * lci], f32, tag="affsL")
                nc.scalar.activation(affs_last[:],
                                     aff_ps[:, lcoff : lcoff + JB * lci],
                                     Ident, bias=bb_sb)

            if il == ci - 1:
                # ---- chunk c: aff matmuls, sign/affinity chain ----
                csl = JB * ci
                coff = JB * CH_OFF[c]
                last = (c == len(CHUNKS) - 1)
                with tc.high_priority():
                    if not last:
                        for b in range(JB):
                            ao = coff + b * ci
                            nc.tensor.matmul(aff_ps[:, ao : ao + ci],
                                             xt_sb[:, b * H : (b + 1) * H],
                                             y2_sb[:, CH_OFF[c] : CH_OFF[c] + ci],
                                             start=True, stop=True)
                    pq_c = pq_ps[:, 2 * coff : 2 * (coff + csl)].rearrange(
                        "p (x two) -> p x two", two=2)
                    p_v = pq_c[:, :, 0:1]
                    q_v = pq_c[:, :, 1:2]
                    if last:
                        # all-DVE direct-PSUM chain; aff already drained (+bb)
                        m2 = work.tile([H, csl], f32, tag=f"m2{c}")
                        m23 = m2[:].rearrange("p (x one) -> p x one", one=1)
                        nc.vector.tensor_scalar(m23, p_v, pb_sb, 0.0,
                                                Alu.add, Alu.min)
                        hp1 = work.tile([H, csl], f32, tag=f"hp1{c}")
                        hp13 = hp1[:].rearrange("p (x one) -> p x one", one=1)
                        nc.vector.tensor_scalar(hp13, p_v, npb_sb, 1.0,
                                                Alu.is_ge, Alu.add)
                        r = work.tile([H, csl], f32, tag=f"r{c}")
                        r3 = r[:].rearrange("p (x one) -> p x one", one=1)
                        nc.vector.scalar_tensor_tensor(r3, q_v, qb_sb, m23,
                                                       Alu.add, Alu.subtract)
                        g2 = work.tile([H, csl], f32, tag=f"g2{c}")
                        nc.vector.scalar_tensor_tensor(g2[:], r[:], 0.0, hp1[:],
                                                       Alu.is_ge, Alu.mult)
                        reg = work.tile([H, csl], f32, tag=f"reg{c}")
                        nc.vector.scalar_tensor_tensor(reg[:], g2[:], 1.0,
                                                       affs_last[:],
                                                       Alu.subtract, Alu.mult)
                    else:
                        # ONE unbiased ACT deint of the interleaved [P|Q]
                        # block; biases fold into the Pool chain's scalars
                        PQ = work.tile([H, 2 * csl], f32, tag=f"PQ{c}")
                        nc.scalar.activation(PQ[:], pq_ps[:, 2 * coff : 2 * (coff + csl)],
                                             Ident, bias=0.0)
                        PQ3 = PQ[:].rearrange("p (x two) -> p x two", two=2)
                        P_v = PQ3[:, :, 0:1]
                        Q_v = PQ3[:, :, 1:2]
                        m2 = work.tile([H, csl], f32, tag=f"m2{c}")
                        m23 = m2[:].rearrange("p (x one) -> p x one", one=1)
                        hp1 = work.tile([H, csl], f32, tag=f"hp1{c}")
                        hp13 = hp1[:].rearrange("p (x one) -> p x one", one=1)
                        rq = work.tile([H, csl], f32, tag=f"rq{c}")
                        rq3 = rq[:].rearrange("p (x one) -> p x one", one=1)
                        gb = work.tile([H, csl], f32, tag=f"gb{c}")
                        g2 = work.tile([H, csl], f32, tag=f"g2{c}")
                        s2 = work.tile([H, csl], f32, tag=f"s2{c}")
                        affs = work.tile([H, csl], f32, tag=f"affs{c}")
                        reg = work.tile([H, csl], f32, tag=f"reg{c}")
                        # m2 = min(P+pb, 0)
                        nc.gpsimd.tensor_scalar(m23, P_v, pb_sb, 0.0,
                                                Alu.add, Alu.min)
                        # hp1 = 1[P >= -pb] + 1
                        nc.gpsimd.tensor_scalar(hp13, P_v, npb_sb, 1.0,
                                                Alu.is_ge, Alu.add)
                        # rq = Q - m2   (true r = rq + qb)
                        nc.gpsimd.tensor_tensor(rq3, Q_v, m23, Alu.subtract)
                        # gb = 1[rq >= -qb]
                        nc.gpsimd.tensor_scalar(gb[:], rq[:], nqb_sb, None,
                                                Alu.is_ge)
                        nc.gpsimd.tensor_tensor(g2[:], gb[:], hp1[:], Alu.mult)
                        nc.gpsimd.tensor_scalar(s2[:], g2[:], 1.0, None,
                                                Alu.subtract)
                        nc.scalar.activation(affs[:], aff_ps[:, coff : coff + csl],
                                             Ident, bias=bb_sb)
                        nc.gpsimd.tensor_tensor(reg[:], s2[:], affs[:], Alu.mult)
                    # output DMA for this chunk: [j,(b,i)] -> outT[b*H+j, off+i]
                    dst = outT[:, CH_OFF[c] : CH_OFF[c] + ci].rearrange(
                        "(b j) i -> j b i", b=JB)
                    src = reg[:].rearrange("p (b i) -> p b i", b=JB)
                    nc.sync.dma_start(dst, src)

    try:
        nc._tile_perfetto = list(tc._perfetto_entries)
    except Exception:
        nc._tile_perfetto = []
    nc.compile()
    _NC_CACHE[key] = nc
    return nc


def make_in_maps(inputs):
    X = np.ascontiguousarray(np.asarray(inputs["gene_embeddings"], dtype=np.float32))
    W1 = np.asarray(inputs["W1"], dtype=np.float32)
    b1 = np.asarray(inputs["b1"], dtype=np.float32)
    W2 = np.asarray(inputs["W2"], dtype=np.float32)
    b2 = np.asarray(inputs["b2"], dtype=np.float32)
    Wb = np.asarray(inputs["Wb"], dtype=np.float32)
    bb = np.asarray(inputs["bb"], dtype=np.float32)

    XT = np.ascontiguousarray(X.T)  # [H, N]
    u = W2[0] - W2[1]
    v = W2[0] - W2[2]
    pb = float(b2[0] - b2[1])
    qb = float(b2[0] - b2[2])

    # host-side preamble: Bj (b1 folded), per-core Ai and y2
    bjT = (X @ W1[:, H:].T + b1).T.astype(np.float32)         # [H, N]
    uv = np.stack([u, v], axis=1).astype(np.float32)          # [H, 2]
    bj16 = np.empty((H, N + 2), dtype=np.float16)
    bj16[:, 0:N] = bjT.astype(np.float16)
    bj16[:, N : N + 2] = uv.astype(np.float16)

    aiT_full = (X @ W1[:, :H].T).T.astype(np.float32)         # [H, N]
    y2_full = (Wb[0].T @ XT).astype(np.float32)               # [H, N]

    in_maps = []
    for c in range(NCORES):
        sl = slice(c * R, (c + 1) * R)
        fa = np.empty((H, R + 7), dtype=np.float32)
        fa[:, 0:R] = aiT_full[:, sl]
        fa[:, R] = pb
        fa[:, R + 1] = -pb
        fa[:, R + 2] = qb
        fa[:, R + 3] = -qb
        fa[:, R + 4] = bb[0]
        fa[:, R + 5 : R + 7] = uv
        in_maps.append({
            "bj16": bj16,
            "bj32": bjT,
            "fa": fa,
            "xt": XT,
            "y2": np.ascontiguousarray(y2_full[:, sl]),
        })
    return in_maps


def kernel(**inputs):
    from concourse.bass_utils import run_bass_kernel_spmd

    nc = build_nc()
    in_maps = make_in_maps(inputs)
    res = run_bass_kernel_spmd(nc, in_maps, list(range(NCORES)))
    out = np.empty((N, N), dtype=np.float32)
    for c in range(NCORES):
        out[c * R : (c + 1) * R, :] = res.results[c]["outT"].T
    out[np.arange(N), np.arange(N)] = 0.0
    return out


# revision 7
# speedup vs baseline: 1.8534x; 1.8534x over previous
"""Trainium2 Bass kernel for GeneRegulatoryNetwork pairwise regulatory matrix.

reg[i,j] = sign(argmax(MLP(cat[x_i,x_j]))) * (x_i^T Wb x_j + bb), zero diag.

Decomposition (verified vs reference):
  Ai = X @ W1[:, :h].T            (per-gene i contribution)
  Bj = X @ W1[:, h:].T + b1       (per-gene j contribution, b1 folded)
  hid(i,j) = relu(Ai[i] + Bj[j])               [h]
  p = hid . u ; q = hid . v                    (u = W2[0]-W2[1], v = W2[0]-W2[2])
  Sign closed form (matches first-max argmax semantics exactly), with
  P = p+pb, Q = q+qb (pb = b2[0]-b2[1], qb = b2[0]-b2[2]):
      m2  = min(P, 0) ; hp1 = 1[P >= 0] + 1
      r   = Q - m2                   (r >= 0  <=>  NOT class2)
      g2  = 1[r >= 0] * hp1          (in {0, 1, 2})
      reg = (g2 - 1) * (aff + bb)
  aff[j,i] = xt[:,jblk].T @ y2  with y2 = Wb0.T @ Xm.T (host-precomputed)

v3 design (cost-model driven):
  - HOST precomputes bjT (fp16 + fp32, b1 folded), aiT, y2: no device
    preamble matmuls/drains.  Device work = 96 hid ops + tiny PE matmuls
    + postprocess.
  - hid row split D/A/G ~ 60/15/21 balances engine busy (260/825/640 ns
    per row).  DVE + Pool rows read fp16 bjT (DVE 4x mode); ACT rows read
    fp32 bjT (dtype-neutral cost) to keep the rel-err margin.
  - Non-last chunks: ONE unbiased ACT deint of the interleaved P/Q PSUM
    block (biases folded into the Pool chain's scalar slots) + ACT affs
    drain + 7-op Pool chain.  DVE does nothing here.
  - Last chunk: all-D rows + all-DVE direct-PSUM chain (shortest tail).
  - Postprocess + output DMAs emitted under tc.high_priority() so ready
    chain ops preempt queued hid ops (kills the chunk-post latency drift).
  - Inputs staged over 3 parallel engine DMA queues (SP / ACT / Pool).
"""

import os as _os
import sys

if "/opt/trn_rl_repo" not in sys.path:
    sys.path.insert(0, "/opt/trn_rl_repo")

import numpy as np

N = 768
H = 128
NCORES = 8
R = N // NCORES  # 96 rows per core
JB = N // H      # 6 j-blocks of 128
S = JB * R       # 576 (b, i) slots

# i-chunk sizes for postprocess (last chunk small + all-DVE -> short tail)
CHUNKS = [int(x) for x in _os.environ.get("BASS_CHUNKS", "23,23,23,23,4").split(",")]
assert sum(CHUNKS) == R
CH_OFF = [sum(CHUNKS[:k]) for k in range(len(CHUNKS))]
# engine split for the hid ops (DVE / ACT / GPSIMD; DVE gets the rest)
ND_A = int(_os.environ.get("BASS_NA", "16"))
ND_G = int(_os.environ.get("BASS_NG", "20"))
# how many of the A / G rows read the fp16 bjT (earliest ones, so ACT/Pool
# can start before the fp32 bjT lands); the rest read fp32 for accuracy
A16 = int(_os.environ.get("BASS_A16", "1"))
G16 = int(_os.environ.get("BASS_G16", "2"))

_NC_CACHE = {}


def _engine_pattern():
    """Static i -> engine map from {"D", "A", "G"}.

    Weighted interleave over chunks 0..n-2 (grouping by engine serializes
    the schedule: the hid tile pool hands out slots in emission order).
    Each chunk's last two rows are forced to D so the chunk's final pq
    lands quickly and its postprocess isn't gated on an 825ns ACT op.
    The last chunk is all-D: its rows + the tail chain run back-to-back
    on DVE while ACT/Pool finish the previous chunk's postprocess.
    """
    lci = CHUNKS[-1]
    body = R - lci
    nd0 = body - ND_A - ND_G
    assert nd0 >= 0
    counts = {"A": ND_A, "G": ND_G, "D": nd0}
    acc = {"A": 0.0, "G": 0.0, "D": 0.0}
    pat = []
    for _ in range(body):
        for e in counts:
            acc[e] += counts[e] / body
        e = max(acc, key=lambda k: acc[k])
        acc[e] -= 1.0
        pat.append(e)
    for c in range(len(CHUNKS) - 1):
        lo, hi = CH_OFF[c], CH_OFF[c] + CHUNKS[c]
        for k in (hi - 1, hi - 2):
            if pat[k] != "D":
                for m in range(hi - 3, lo - 1, -1):
                    if pat[m] == "D":
                        pat[m], pat[k] = pat[k], pat[m]
                        break
    return pat + ["D"] * lci


def build_nc():
    key = (ND_A, ND_G, tuple(CHUNKS))
    if key in _NC_CACHE:
        return _NC_CACHE[key]
    from contextlib import ExitStack

    import concourse.bass as bass
    import concourse.tile as tile
    from concourse import bacc, mybir

    f32 = mybir.dt.float32
    fp16 = mybir.dt.float16
    Alu = mybir.AluOpType
    Relu = mybir.ActivationFunctionType.Relu
    Ident = mybir.ActivationFunctionType.Identity

    nc = bacc.Bacc("TRN2", target_bir_lowering=False, debug=False)

    # bj16: [bjT fp16 (768) | uv fp16 (2)]
    d_bj = nc.dram_tensor("bj16", [H, N + 2], fp16, kind="ExternalInput").ap()
    # fa: [aiT f32 (96) | pbc | npbc | qbc | nqbc | bbc | uv32 (2)]
    FA_W = R + 5 + 2
    d_fa = nc.dram_tensor("fa", [H, FA_W], f32, kind="ExternalInput").ap()
    d_bj32 = nc.dram_tensor("bj32", [H, N], f32, kind="ExternalInput").ap()
    d_xt = nc.dram_tensor("xt", [H, N], f32, kind="ExternalInput").ap()
    d_y2 = nc.dram_tensor("y2", [H, R], f32, kind="ExternalInput").ap()
    outT = nc.dram_tensor("outT", [N, R], f32, kind="ExternalOutput").ap()

    pat = _engine_pattern()

    with tile.TileContext(nc) as tc, ExitStack() as ctx:
        const = ctx.enter_context(tc.tile_pool(name="const", bufs=1))
        work = ctx.enter_context(tc.tile_pool(name="work", bufs=1))
        hidp = ctx.enter_context(
            tc.tile_pool(name="hid", bufs=int(_os.environ.get("BASS_HBUF", "20"))))
        pspq = ctx.enter_context(tc.tile_pool(name="pspq", bufs=1, space="PSUM"))
        psaf = ctx.enter_context(tc.tile_pool(name="psaf", bufs=1, space="PSUM"))

        pq_ps = pspq.tile([H, 2 * S], f32, tag="pq")       # [j, (c, b, i, 2)]
        aff_ps = psaf.tile([H, S], f32, tag="aff")          # [j, (c, b, i)]

        bj_sb = const.tile([H, N + 2], fp16, tag="bj")
        fa_sb = const.tile([H, FA_W], f32, tag="fa")
        bj32_sb = const.tile([H, N], f32, tag="bj32")
        xt_sb = const.tile([H, N], f32, tag="xt")
        y2_sb = const.tile([H, R], f32, tag="y2")

        # ---- input DMAs on 3 parallel engine queues (SP / ACT / Pool) ----
        nc.sync.dma_start(bj_sb[:], d_bj[:])                      # arr ~2.31us
        nc.scalar.dma_start(fa_sb[:], d_fa[:])                    # arr ~2.22us
        nc.scalar.dma_start(bj32_sb[:, 0:384], d_bj32[:, 0:384])  # arr ~2.81us
        nc.gpsimd.dma_start(bj32_sb[:, 384:N], d_bj32[:, 384:N])  # arr ~2.48us
        nc.sync.dma_start(xt_sb[:], d_xt[:])                      # arr ~3.50us
        nc.gpsimd.dma_start(y2_sb[:], d_y2[:])                    # arr ~2.98us

        # ---- t=0: trigger the ACT table load during the DMA wait ----
        tw = const.tile([H, 1], f32, tag="tw")
        nc.vector.memset(tw[:], 0.25)
        tact = const.tile([H, 1], f32, tag="tact")
        nc.scalar.activation(tact[:], tw[:], Relu, bias=0.0)

        bjT16 = bj_sb[:, 0:N]
        uv16 = bj_sb[:, N : N + 2]
        aiT = fa_sb[:, 0:R]
        pb_sb = fa_sb[:, R : R + 1]
        npb_sb = fa_sb[:, R + 1 : R + 2]
        qb_sb = fa_sb[:, R + 2 : R + 3]
        nqb_sb = fa_sb[:, R + 3 : R + 4]
        bb_sb = fa_sb[:, R + 4 : R + 5]
        uv32 = fa_sb[:, R + 5 : R + 7]

        # ---- main loop ----
        affs_last = None
        c = 0
        na16 = A16
        ng16 = G16
        for i in range(R):
            while i >= CH_OFF[c] + CHUNKS[c]:
                c += 1
            il = i - CH_OFF[c]
            ci = CHUNKS[c]
            e = pat[i]
            if e == "A":
                if na16 > 0:
                    na16 -= 1
                    hid = hidp.tile([H, N], fp16, tag="hid")
                    nc.scalar.activation(hid[:], bjT16, Relu,
                                         bias=aiT[:, i : i + 1])
                    uv_mm = uv16
                else:
                    hid = hidp.tile([H, N], f32, tag="hid")
                    nc.scalar.activation(hid[:], bj32_sb[:], Relu,
                                         bias=aiT[:, i : i + 1])
                    uv_mm = uv32
            elif e == "G":
                if ng16 > 0:
                    ng16 -= 1
                    hid = hidp.tile([H, N], fp16, tag="hid")
                    nc.gpsimd.tensor_scalar(hid[:], bjT16, aiT[:, i : i + 1],
                                            0.0, Alu.add, Alu.max)
                    uv_mm = uv16
                else:
                    hid = hidp.tile([H, N], f32, tag="hid")
                    nc.gpsimd.tensor_scalar(hid[:], bj32_sb[:], aiT[:, i : i + 1],
                                            0.0, Alu.add, Alu.max)
                    uv_mm = uv32
            else:
                hid = hidp.tile([H, N], fp16, tag="hid")
                nc.vector.tensor_scalar(hid[:], bjT16, aiT[:, i : i + 1],
                                        0.0, Alu.add, Alu.max)
                uv_mm = uv16
            for b in range(JB):
                o = 2 * (JB * CH_OFF[c] + b * ci + il)
                nc.tensor.matmul(pq_ps[:, o : o + 2], hid[:, b * H : (b + 1) * H],
                                 uv_mm, start=True, stop=True)

            if i == CH_OFF[2]:
                # last chunk's affinity (+bb) computed mid-loop: PE and ACT
                # both have slack here, so the final chunk's reg op only
                # needs cheap SBUF stt's in the tail
                lc = len(CHUNKS) - 1
                lci = CHUNKS[lc]
                lcoff = JB * CH_OFF[lc]
                for b in range(JB):
                    ao = lcoff + b * lci
                    nc.tensor.matmul(aff_ps[:, ao : ao + lci],
                                     xt_sb[:, b * H : (b + 1) * H],
                                     y2_sb[:, CH_OFF[lc] : CH_OFF[lc] + lci],
                                     start=True, stop=True)
                affs_last = work.tile([H, JB * lci], f32, tag="affsL")
                nc.scalar.activation(affs_last[:],
                                     aff_ps[:, lcoff : lcoff + JB * lci],
                                     Ident, bias=bb_sb)

            if il == ci - 1:
                # ---- chunk c: aff matmuls, sign/affinity chain ----
                csl = JB * ci
                coff = JB * CH_OFF[c]
                last = (c == len(CHUNKS) - 1)
                with tc.high_priority():
                    if not last:
                        for b in range(JB):
                            ao = coff + b * ci
                            nc.tensor.matmul(aff_ps[:, ao : ao + ci],
                                             xt_sb[:, b * H : (b + 1) * H],
                                             y2_sb[:, CH_OFF[c] : CH_OFF[c] + ci],
                                             start=True, stop=True)
                    pq_c = pq_ps[:, 2 * coff : 2 * (coff + csl)].rearrange(
                        "p (x two) -> p x two", two=2)
                    p_v = pq_c[:, :, 0:1]
                    q_v = pq_c[:, :, 1:2]
                    if last:
                        # all-DVE direct-PSUM chain; aff already drained (+bb)
                        m2 = work.tile([H, csl], f32, tag=f"m2{c}")
                        m23 = m2[:].rearrange("p (x one) -> p x one", one=1)
                        nc.vector.tensor_scalar(m23, p_v, pb_sb, 0.0,
                                                Alu.add, Alu.min)
                        hp1 = work.tile([H, csl], f32, tag=f"hp1{c}")
                        hp13 = hp1[:].rearrange("p (x one) -> p x one", one=1)
                        nc.vector.tensor_scalar(hp13, p_v, npb_sb, 1.0,
                                                Alu.is_ge, Alu.add)
                        r = work.tile([H, csl], f32, tag=f"r{c}")
                        r3 = r[:].rearrange("p (x one) -> p x one", one=1)
                        nc.vector.scalar_tensor_tensor(r3, q_v, qb_sb, m23,
                                                       Alu.add, Alu.subtract)
                        g2 = work.tile([H, csl], f32, tag=f"g2{c}")
                        nc.vector.scalar_tensor_tensor(g2[:], r[:], 0.0, hp1[:],
                                                       Alu.is_ge, Alu.mult)
                        reg = work.tile([H, csl], f32, tag=f"reg{c}")
                        nc.vector.scalar_tensor_tensor(reg[:], g2[:], 1.0,
                                                       affs_last[:],
                                                       Alu.subtract, Alu.mult)
                    else:
                        # ONE unbiased ACT deint of the interleaved [P|Q]
                        # block; biases fold into the Pool chain's scalars
                        PQ = work.tile([H, 2 * csl], f32, tag=f"PQ{c}")
                        nc.scalar.activation(PQ[:], pq_ps[:, 2 * coff : 2 * (coff + csl)],
                                             Ident, bias=0.0)
                        PQ3 = PQ[:].rearrange("p (x two) -> p x two", two=2)
                        P_v = PQ3[:, :, 0:1]
                        Q_v = PQ3[:, :, 1:2]
                        m2 = work.tile([H, csl], f32, tag=f"m2{c}")
                        m23 = m2[:].rearrange("p (x one) -> p x one", one=1)
                        hp1 = work.tile([H, csl], f32, tag=f"hp1{c}")
                        hp13 = hp1[:].rearrange("p (x one) -> p x one", one=1)
                        rq = work.tile([H, csl], f32, tag=f"rq{c}")
                        rq3 = rq[:].rearrange("p (x one) -> p x one", one=1)
                        gb = work.tile([H, csl], f32, tag=f"gb{c}")
                        g2 = work.tile([H, csl], f32, tag=f"g2{c}")
                        s2 = work.tile([H, csl], f32, tag=f"s2{c}")
                        affs = work.tile([H, csl], f32, tag=f"affs{c}")
                        reg = work.tile([H, csl], f32, tag=f"reg{c}")
                        # m2 = min(P+pb, 0)
                        nc.gpsimd.tensor_scalar(m23, P_v, pb_sb, 0.0,
                                                Alu.add, Alu.min)
                        # hp1 = 1[P >= -pb] + 1
                        nc.gpsimd.tensor_scalar(hp13, P_v, npb_sb, 1.0,
                                                Alu.is_ge, Alu.add)
                        # rq = Q - m2   (true r = rq + qb)
                        nc.gpsimd.tensor_tensor(rq3, Q_v, m23, Alu.subtract)
                        # gb = 1[rq >= -qb]
                        nc.gpsimd.tensor_scalar(gb[:], rq[:], nqb_sb, None,
                                                Alu.is_ge)
                        nc.gpsimd.tensor_tensor(g2[:], gb[:], hp1[:], Alu.mult)
                        nc.gpsimd.tensor_scalar(s2[:], g2[:], 1.0, None,
                                                Alu.subtract)
                        nc.scalar.activation(affs[:], aff_ps[:, coff : coff + csl],
                                             Ident, bias=bb_sb)
                        nc.gpsimd.tensor_tensor(reg[:], s2[:], affs[:], Alu.mult)
                    # output DMA for this chunk: [j,(b,i)] -> outT[b*H+j, off+i]
                    dst = outT[:, CH_OFF[c] : CH_OFF[c] + ci].rearrange(
                        "(b j) i -> j b i", b=JB)
                    src = reg[:].rearrange("p (b i) -> p b i", b=JB)
                    nc.sync.dma_start(dst, src)

    try:
        nc._tile_perfetto = list(tc._perfetto_entries)
    except Exception:
        nc._tile_perfetto = []
    nc.compile()
    _NC_CACHE[key] = nc
    return nc


def make_in_maps(inputs):
    X = np.ascontiguousarray(np.asarray(inputs["gene_embeddings"], dtype=np.float32))
    W1 = np.asarray(inputs["W1"], dtype=np.float32)
    b1 = np.asarray(inputs["b1"], dtype=np.float32)
    W2 = np.asarray(inputs["W2"], dtype=np.float32)
    b2 = np.asarray(inputs["b2"], dtype=np.float32)
    Wb = np.asarray(inputs["Wb"], dtype=np.float32)
    bb = np.asarray(inputs["bb"], dtype=np.float32)

    XT = np.ascontiguousarray(X.T)  # [H, N]
    u = W2[0] - W2[1]
    v = W2[0] - W2[2]
    pb = float(b2[0] - b2[1])
    qb = float(b2[0] - b2[2])

    # host-side preamble: Bj (b1 folded), per-core Ai and y2
    bjT = (X @ W1[:, H:].T + b1).T.astype(np.float32)         # [H, N]
    uv = np.stack([u, v], axis=1).astype(np.float32)          # [H, 2]
    bj16 = np.empty((H, N + 2), dtype=np.float16)
    bj16[:, 0:N] = bjT.astype(np.float16)
    bj16[:, N : N + 2] = uv.astype(np.float16)

    aiT_full = (X @ W1[:, :H].T).T.astype(np.float32)         # [H, N]
    y2_full = (Wb[0].T @ XT).astype(np.float32)               # [H, N]

    in_maps = []
    for c in range(NCORES):
        sl = slice(c * R, (c + 1) * R)
        fa = np.empty((H, R + 7), dtype=np.float32)
        fa[:, 0:R] = aiT_full[:, sl]
        fa[:, R] = pb
        fa[:, R + 1] = -pb
        fa[:, R + 2] = qb
        fa[:, R + 3] = -qb
        fa[:, R + 4] = bb[0]
        fa[:, R + 5 : R + 7] = uv
        in_maps.append({
            "bj16": bj16,
            "bj32": bjT,
            "fa": fa,
            "xt": XT,
            "y2": np.ascontiguousarray(y2_full[:, sl]),
        })
    return in_maps


def kernel(**inputs):
    from concourse.bass_utils import run_bass_kernel_spmd

    nc = build_nc()
    in_maps = make_in_maps(inputs)
    res = run_bass_kernel_spmd(nc, in_maps, list(range(NCORES)))
    out = np.empty((N, N), dtype=np.float32)
    for c in range(NCORES):
        out[c * R : (c + 1) * R, :] = res.results[c]["outT"].T
    out[np.arange(N), np.arange(N)] = 0.0
    return out
